# revision 16
# baseline (speedup 1.0000x reference)
"""Two-layer GCN (PyG GCNConv defaults) on 8 Trainium2 NeuronCores.

Strategy (graph/data parallel, per sharding hint):
  - Destination nodes sharded 8 x 12544 (N=100000 padded to 100352).
  - Per layer: g = deg^{-1/2} * (h @ W) computed on each core's own shard,
    then AllGather'd (bf16) so every core holds the full g table in DRAM.
  - Edge aggregation per core: edges with dst in the core's shard, sorted by
    (src-bucket, dst-block).  Source features fetched with dma_gather
    (int16 local indices into one of 4 src-range buckets of 25088 rows),
    with the 4 buckets' gathers running on 4 SWDGE queues concurrently.
  - segment_sum realized as selector matmuls: S.T[e, d] = (dstloc[e] == d)
    built on-chip via is_equal against an iota tile; each dst block's four
    bucket-segments accumulate directly in one PSUM slice, finalized in place:
    out = deg^{-1/2} * (segsum + g_own) + b, LeakyReLU(0.01).  Layer 2 reuses
    the same edge structure against the layer-2 g table.
"""
import sys

sys.path.insert(0, "/opt/trn_rl_repo")

import numpy as np
import ml_dtypes

import concourse.bacc as bacc
import concourse.mybir as mybir
import concourse.tile as tile
from concourse.bass_utils import run_bass_kernel_spmd

NCORES = 8
N = 100000
E = 1600000
D = 128
SH = 12544              # dst shard per core (= 98 blocks of 128)
NP_ = SH * NCORES       # padded node count 100352
BLK = SH // 128         # 98 dst blocks per core
NB = 4                  # src buckets (int16 index limit 32767 >= 25088)
BKT = NP_ // NB         # 25088 rows per bucket table
PIECE = 7               # dst blocks per dma_gather piece (98 = 14 * 7)
NPIECE = BLK // PIECE   # 14 pieces per bucket
NEG = 0.01

fp32 = mybir.dt.float32
bf16 = mybir.dt.bfloat16
i16 = mybir.dt.int16
i32 = mybir.dt.int32

_CACHE = {}


def _build(cap):
    ck = cap // 128                 # chunks per segment
    ts = NB * BLK * cap             # idx stream length per core / layer
    nch = ts // 128                 # total chunks in stream
    pidx = PIECE * cap              # idxs per gather piece

    nc = bacc.Bacc("TRN2", num_devices=NCORES, num_swdge_queues=4)
    xT_in = nc.dram_tensor("xT", [128, SH], fp32, kind="ExternalInput")
    w1_in = nc.dram_tensor("w1", [128, 128], fp32, kind="ExternalInput")
    w2_in = nc.dram_tensor("w2", [128, 128], fp32, kind="ExternalInput")
    b1_in = nc.dram_tensor("b1t", [128, 128], fp32, kind="ExternalInput")
    b2_in = nc.dram_tensor("b2t", [128, 128], fp32, kind="ExternalInput")
    cnt_in = nc.dram_tensor("cnt", [128, BLK], i32, kind="ExternalInput")
    cntf_in = nc.dram_tensor("cntf", [1, SH], i32, kind="ExternalInput")
    idx_in = nc.dram_tensor("idx", [128, ts // 16], i16, kind="ExternalInput")
    dl_in = nc.dram_tensor("dstloc", [128, nch], fp32, kind="ExternalInput")
    iota_in = nc.dram_tensor("iota", [128, cap], bf16, kind="ExternalInput")
    ident_in = nc.dram_tensor("ident", [128, 128], bf16, kind="ExternalInput")
    out_t = nc.dram_tensor("out", [SH, D], fp32, kind="ExternalOutput")

    with tile.TileContext(nc) as tc:
        with (
            tc.tile_pool(name="const", bufs=1) as cpool,
            tc.tile_pool(name="xchunk", bufs=3) as xpool,
            tc.tile_pool(name="msg", bufs=3) as mpool,
            tc.tile_pool(name="st", bufs=4) as stpool,
            tc.tile_pool(name="ix", bufs=2) as ixpool,
            tc.tile_pool(name="fin", bufs=3) as fpool,
            tc.tile_pool(name="ps_w", bufs=2, space="PSUM") as ps_w,
            tc.tile_pool(name="ps_seg", bufs=4, space="PSUM") as ps_seg,
            tc.tile_pool(name="ps_t", bufs=2, space="PSUM") as ps_t,
            tc.tile_pool(name="dram", bufs=1, space="DRAM") as dram,
        ):
            # ---- resident constants -------------------------------------
            w1f = cpool.tile([128, 128], fp32)
            nc.sync.dma_start(w1f[:], w1_in[:])
            w1b = cpool.tile([128, 128], bf16)
            nc.vector.tensor_copy(w1b[:], w1f[:])
            w2f = cpool.tile([128, 128], fp32)
            nc.sync.dma_start(w2f[:], w2_in[:])
            w2b = cpool.tile([128, 128], bf16)
            nc.vector.tensor_copy(w2b[:], w2f[:])
            b1t = cpool.tile([128, 128], fp32)
            nc.sync.dma_start(b1t[:], b1_in[:])
            b2t = cpool.tile([128, 128], fp32)
            nc.sync.dma_start(b2t[:], b2_in[:])
            iota = cpool.tile([128, cap], bf16)
            nc.sync.dma_start(iota[:], iota_in[:])
            ident = cpool.tile([128, 128], bf16)
            nc.sync.dma_start(ident[:], ident_in[:])
            dstloc = cpool.tile([128, nch], fp32)
            nc.sync.dma_start(dstloc[:], dl_in[:])

            # deg^{-1/2} from int32 counts:  1 / sqrt(cnt + 1)
            cnts = cpool.tile([128, BLK], i32)
            nc.sync.dma_start(cnts[:], cnt_in[:])
            degf = cpool.tile([128, BLK], fp32)
            nc.vector.tensor_copy(degf[:], cnts[:])
            sq = cpool.tile([128, BLK], fp32)
            nc.scalar.activation(sq[:], degf[:], mybir.ActivationFunctionType.Sqrt,
                                 bias=1.0)
            dq = cpool.tile([128, BLK], fp32)
            nc.vector.reciprocal(dq[:], sq[:])

            # flat sqrt(deg) [1, SH] for the rank-1 bias fold (slice at
            # partition 0), plus b as [1,128] bf16 rows
            sqT = cpool.tile([1, SH], bf16)
            with tc.tile_pool(name="tmpf", bufs=1) as tpool:
                fw = SH // 8
                for j in range(8):
                    cf = tpool.tile([1, fw], i32, tag="cf")
                    nc.sync.dma_start(cf[:], cntf_in[0:1, j * fw:(j + 1) * fw])
                    dg = tpool.tile([1, fw], fp32, tag="dg")
                    nc.vector.tensor_copy(dg[:], cf[:])
                    nc.scalar.activation(sqT[0:1, j * fw:(j + 1) * fw], dg[:],
                                         mybir.ActivationFunctionType.Sqrt,
                                         bias=1.0)
            b1r = cpool.tile([1, 128], bf16)
            nc.vector.tensor_copy(b1r[:], b1t[0:1, :])
            b2r = cpool.tile([1, 128], bf16)
            nc.vector.tensor_copy(b2r[:], b2t[0:1, :])

            # DRAM bounce + gathered tables
            g1_b = dram.tile([SH, D], bf16, name="g1_b")
            g1_full = dram.tile([NP_, D], bf16, name="g1_full")
            g2_b = dram.tile([SH, D], bf16, name="g2_b")
            g2_full = dram.tile([NP_, D], bf16, name="g2_full")

            def own_piece(g_b, p):
                t = fpool.tile([128, PIECE, 128], bf16, tag="gown")
                nc.sync.dma_start(
                    t[:], g_b[:].rearrange("(a p) d -> p a d", p=128)
                    [:, p * PIECE:(p + 1) * PIECE, :])
                return t

            # ---- phase A: g1 = dq * (x @ W1) on own shard ---------------
            with nc.named_scope("phaseA"):
                for pc in range(NPIECE):
                    c0 = pc * PIECE
                    xc = xpool.tile([128, PIECE * 128], fp32, tag="xc")
                    nc.sync.dma_start(
                        xc[:], xT_in[:, c0 * 128:(c0 + PIECE) * 128])
                    xcb = xpool.tile([128, PIECE * 128], bf16, tag="xcb")
                    nc.vector.tensor_copy(xcb[:], xc[:])
                    gt = xpool.tile([128, PIECE, 128], bf16, tag="gt")
                    for j in range(PIECE):
                        ph = ps_w.tile([128, 128], fp32, space="PSUM", tag="ph")
                        nc.tensor.matmul(
                            out=ph[:], lhsT=xcb[:, j * 128:(j + 1) * 128],
                            rhs=w1b[:], start=True, stop=True)
                        nc.scalar.activation(
                            gt[:, j, :], ph[:],
                            mybir.ActivationFunctionType.Copy,
                            scale=dq[:, c0 + j:c0 + j + 1])
                    nc.sync.dma_start(
                        g1_b[:].rearrange("(a p) d -> p a d", p=128)
                        [:, c0:c0 + PIECE, :], gt[:])

            with nc.named_scope("ag1"):
                nc.gpsimd.collective_compute(
                    "AllGather", mybir.AluOpType.bypass,
                    replica_groups=[list(range(NCORES))],
                    ins=[g1_b[:].opt()], outs=[g1_full[:].opt()])

            # ---- aggregation over edges (shared for both layers) --------
            def aggregate(g_full, g_b, b_row, finalize, stage_dt, flush):
                for p in range(NPIECE):
                    msgs = []
                    for r in range(NB):
                        off = (r * BLK + p * PIECE) * cap
                        msg = mpool.tile([128, pidx // 128, 128], bf16,
                                         tag=f"msg{r}")
                        ix = ixpool.tile([128, pidx // 16], i16, tag=f"ix{r}")
                        nc.sync.dma_start(
                            ix[:], idx_in[:, off // 16:(off + pidx) // 16])
                        nc.gpsimd.dma_gather(
                            msg[:], g_full[r * BKT:(r + 1) * BKT, :],
                            ix[:], pidx, pidx, 128, single_packet=False,
                            queue_num=r)
                        msgs.append(msg)
                    gown = own_piece(g_b, p)
                    stage = fpool.tile([128, PIECE, 128], stage_dt, tag="stage")
                    for s in range(PIECE):
                        c = p * PIECE + s
                        pseg = ps_seg.tile([128, 128], fp32, space="PSUM",
                                           tag="pseg")
                        for r in range(NB):
                            gc0 = (r * BLK + p * PIECE) * cap // 128 + s * ck
                            st = stpool.tile([128, cap], bf16, tag="st")
                            nc.vector.tensor_tensor(
                                st[:].rearrange("q (c j) -> q c j", c=ck),
                                iota[:].rearrange("q (c j) -> q c j", c=ck),
                                dstloc[:, gc0:gc0 + ck].to_broadcast(
                                    [128, ck, 128]),
                                mybir.AluOpType.is_equal)
                            for k in range(ck):
                                nc.tensor.matmul(
                                    out=pseg[:],
                                    lhsT=st[:, k * 128:(k + 1) * 128],
                                    rhs=msgs[r][:, s * ck + k, :],
                                    start=(r == 0 and k == 0),
                                    stop=False)
                        # self-loop: pseg += I.T @ g_own ;  bias (pre-scale):
                        # pseg += sqrt(deg) (x) b  so that (pseg)*dq = out+b
                        nc.tensor.matmul(out=pseg[:], lhsT=ident[:],
                                         rhs=gown[:, s, :],
                                         start=False, stop=False)
                        nc.tensor.matmul(out=pseg[:], lhsT=sqT[0:1, c * 128:(c + 1) * 128],
                                         rhs=b_row[:],
                                         start=False, stop=True)
                        finalize(c, pseg[:], stage[:, s, :])
                    flush(p, stage)

            # ---- layer-1 finalize: h2 path, produces g2 -----------------
            def fin1(c, pslice, sl):
                r1 = fpool.tile([128, 128], bf16, tag="r1")
                nc.scalar.activation(r1[:], pslice,
                                     mybir.ActivationFunctionType.Lrelu,
                                     scale=dq[:, c:c + 1], alpha=NEG)
                ptr = ps_t.tile([128, 128], bf16, space="PSUM", tag="ptr")
                nc.tensor.transpose(ptr[:], r1[:], ident[:])
                r1T = fpool.tile([128, 128], bf16, tag="r1T")
                nc.vector.tensor_copy(r1T[:], ptr[:])
                ph2 = ps_w.tile([128, 128], fp32, space="PSUM", tag="ph")
                nc.tensor.matmul(out=ph2[:], lhsT=r1T[:], rhs=w2b[:],
                                 start=True, stop=True)
                nc.scalar.activation(sl, ph2[:],
                                     mybir.ActivationFunctionType.Copy,
                                     scale=dq[:, c:c + 1])

            def flush1(p, stage):
                nc.sync.dma_start(
                    g2_b[:].rearrange("(a p) d -> p a d", p=128)
                    [:, p * PIECE:(p + 1) * PIECE, :], stage[:])

            def fin2(c, pslice, sl):
                nc.scalar.activation(sl, pslice,
                                     mybir.ActivationFunctionType.Lrelu,
                                     scale=dq[:, c:c + 1], alpha=NEG)

            def flush2(p, stage):
                nc.sync.dma_start(
                    out_t[:].rearrange("(a p) d -> p a d", p=128)
                    [:, p * PIECE:(p + 1) * PIECE, :], stage[:])

            with nc.named_scope("agg1"):
                aggregate(g1_full, g1_b, b1r, fin1, bf16, flush1)

            with nc.named_scope("ag2"):
                nc.gpsimd.collective_compute(
                    "AllGather", mybir.AluOpType.bypass,
                    replica_groups=[list(range(NCORES))],
                    ins=[g2_b[:].opt()], outs=[g2_full[:].opt()])

            with nc.named_scope("agg2"):
                aggregate(g2_full, g2_b, b2r, fin2, fp32, flush2)

    nc.compile()
    return nc


def _preprocess(x, edge_index):
    src = np.asarray(edge_index[0], dtype=np.int64)
    dst = np.asarray(edge_index[1], dtype=np.int64)
    core = dst // SH
    block = (dst % SH) // 128
    dstloc = (dst % 128).astype(np.float32)
    bucket = src // BKT
    srcloc = (src % BKT).astype(np.int16)

    seg = (core * NB + bucket) * BLK + block
    counts = np.bincount(seg, minlength=NCORES * NB * BLK)
    cap = max(640, int(-(-counts.max() // 128) * 128))

    order = np.argsort(seg, kind="stable")
    seg_s = seg[order]
    starts = np.zeros(NCORES * NB * BLK + 1, np.int64)
    np.cumsum(counts, out=starts[1:])
    pos = np.arange(E, dtype=np.int64) - starts[seg_s]
    slot = seg_s * cap + pos

    total = NCORES * NB * BLK * cap
    idx_arr = np.zeros(total, np.int16)
    idx_arr[slot] = srcloc[order]
    dl_arr = np.full(total, 255.0, np.float32)
    dl_arr[slot] = dstloc[order]
    ts = NB * BLK * cap
    idx_arr = idx_arr.reshape(NCORES, ts)
    dl_arr = dl_arr.reshape(NCORES, ts)

    cnt = np.bincount(dst, minlength=NP_).astype(np.int32)

    xpad = np.zeros((NP_, D), np.float32)
    xpad[:N] = x

    iota = np.tile(np.arange(128, dtype=np.float32), (128, cap // 128)) \
        .astype(ml_dtypes.bfloat16)
    ident = np.eye(128, dtype=ml_dtypes.bfloat16)

    return cap, idx_arr, dl_arr, cnt, xpad, iota, ident


def kernel(x, W1, b1, W2, b2, edge_index, batch):
    x = np.asarray(x, np.float32)
    W1 = np.asarray(W1, np.float32)
    W2 = np.asarray(W2, np.float32)
    b1 = np.asarray(b1, np.float32)
    b2 = np.asarray(b2, np.float32)

    cap, idx_arr, dl_arr, cnt, xpad, iota, ident = _preprocess(x, edge_index)

    if cap not in _CACHE:
        _CACHE[cap] = _build(cap)
    nc = _CACHE[cap]

    b1t = np.tile(b1, (128, 1))
    b2t = np.tile(b2, (128, 1))
    in_maps = []
    for c in range(NCORES):
        sl = slice(c * SH, (c + 1) * SH)
        wrapped = np.tile(idx_arr[c].reshape(-1, 16).T, (8, 1))
        in_maps.append({
            "xT": np.ascontiguousarray(xpad[sl].T),
            "w1": W1, "w2": W2, "b1t": b1t, "b2t": b2t,
            "cnt": np.ascontiguousarray(cnt[sl].reshape(BLK, 128).T),
            "cntf": np.ascontiguousarray(cnt[sl][None, :]),
            "idx": np.ascontiguousarray(wrapped),
            "dstloc": np.ascontiguousarray(dl_arr[c].reshape(-1, 128).T),
            "iota": iota, "ident": ident,
        })

    import os
    trace = bool(os.environ.get("KERNEL_TRACE"))
    rr = run_bass_kernel_spmd(nc, in_maps, list(range(NCORES)), trace=trace)
    if trace:
        kernel.last_results = rr
    out = np.concatenate([rr.results[c]["out"] for c in range(NCORES)], axis=0)
    return np.ascontiguousarray(out[:N])


# revision 17
# speedup vs baseline: 1.0766x; 1.0766x over previous
"""Two-layer GCN (PyG GCNConv defaults) on 8 Trainium2 NeuronCores.

Strategy (graph/data parallel, per sharding hint):
  - Destination nodes sharded 8 x 12544 (N=100000 padded to 100352).
  - Per layer: g = deg^{-1/2} * (h @ W) computed on each core's own shard,
    then AllGather'd (bf16) so every core holds the full g table in DRAM.
  - Edge aggregation per core: edges with dst in the core's shard, sorted by
    (src-bucket, dst-block).  Source features fetched with dma_gather
    (int16 local indices into one of 4 src-range buckets of 25088 rows),
    with the 4 buckets' gathers running on 4 SWDGE queues concurrently.
  - segment_sum realized as selector matmuls: S.T[e, d] = (dstloc[e] == d)
    built on-chip via is_equal against an iota tile; each dst block's four
    bucket-segments accumulate directly in one PSUM slice, finalized in place:
    out = deg^{-1/2} * (segsum + g_own) + b, LeakyReLU(0.01).  Layer 2 reuses
    the same edge structure against the layer-2 g table.
"""
import sys

sys.path.insert(0, "/opt/trn_rl_repo")

import numpy as np
import ml_dtypes

import concourse.bacc as bacc
import concourse.mybir as mybir
import concourse.tile as tile
from concourse.bass_utils import run_bass_kernel_spmd

NCORES = 8
N = 100000
E = 1600000
D = 128
SH = 12544              # dst shard per core (= 98 blocks of 128)
NP_ = SH * NCORES       # padded node count 100352
BLK = SH // 128         # 98 dst blocks per core
NB = 4                  # src buckets (int16 index limit 32767 >= 25088)
BKT = NP_ // NB         # 25088 rows per bucket table
PIECE = 7               # dst blocks per dma_gather piece (98 = 14 * 7)
NPIECE = BLK // PIECE   # 14 pieces per bucket
NEG = 0.01

fp32 = mybir.dt.float32
bf16 = mybir.dt.bfloat16
i16 = mybir.dt.int16
i32 = mybir.dt.int32

_CACHE = {}


def _build(cap):
    ck = cap // 128                 # chunks per segment
    ts = NB * BLK * cap             # idx stream length per core / layer
    nch = ts // 128                 # total chunks in stream
    pidx = PIECE * cap              # idxs per gather piece

    nc = bacc.Bacc("TRN2", num_devices=NCORES, num_swdge_queues=4)
    xT_in = nc.dram_tensor("xT", [128, SH], fp32, kind="ExternalInput")
    w1_in = nc.dram_tensor("w1", [128, 128], fp32, kind="ExternalInput")
    w2_in = nc.dram_tensor("w2", [128, 128], fp32, kind="ExternalInput")
    b1_in = nc.dram_tensor("b1t", [128, 128], fp32, kind="ExternalInput")
    b2_in = nc.dram_tensor("b2t", [128, 128], fp32, kind="ExternalInput")
    cnt_in = nc.dram_tensor("cnt", [128, BLK], i32, kind="ExternalInput")
    cntf_in = nc.dram_tensor("cntf", [1, SH], i32, kind="ExternalInput")
    idx_in = nc.dram_tensor("idx", [128, ts // 16], i16, kind="ExternalInput")
    dl_in = nc.dram_tensor("dstloc", [128, nch], fp32, kind="ExternalInput")
    iota_in = nc.dram_tensor("iota", [128, cap], bf16, kind="ExternalInput")
    ident_in = nc.dram_tensor("ident", [128, 128], bf16, kind="ExternalInput")
    out_t = nc.dram_tensor("out", [SH, D], fp32, kind="ExternalOutput")

    with tile.TileContext(nc) as tc:
        with (
            tc.tile_pool(name="const", bufs=1) as cpool,
            tc.tile_pool(name="xchunk", bufs=3) as xpool,
            tc.tile_pool(name="msg", bufs=3) as mpool,
            tc.tile_pool(name="st", bufs=4) as stpool,
            tc.tile_pool(name="ix", bufs=2) as ixpool,
            tc.tile_pool(name="fin", bufs=3) as fpool,
            tc.tile_pool(name="ps_w", bufs=2, space="PSUM") as ps_w,
            tc.tile_pool(name="ps_seg", bufs=4, space="PSUM") as ps_seg,
            tc.tile_pool(name="ps_t", bufs=2, space="PSUM") as ps_t,
            tc.tile_pool(name="dram", bufs=1, space="DRAM") as dram,
        ):
            # ---- resident constants -------------------------------------
            w1f = cpool.tile([128, 128], fp32)
            nc.sync.dma_start(w1f[:], w1_in[:])
            w1b = cpool.tile([128, 128], bf16)
            nc.vector.tensor_copy(w1b[:], w1f[:])
            w2f = cpool.tile([128, 128], fp32)
            nc.sync.dma_start(w2f[:], w2_in[:])
            w2b = cpool.tile([128, 128], bf16)
            nc.vector.tensor_copy(w2b[:], w2f[:])
            b1t = cpool.tile([128, 128], fp32)
            nc.sync.dma_start(b1t[:], b1_in[:])
            b2t = cpool.tile([128, 128], fp32)
            nc.sync.dma_start(b2t[:], b2_in[:])
            iota = cpool.tile([128, cap], bf16)
            nc.sync.dma_start(iota[:], iota_in[:])
            ident = cpool.tile([128, 128], bf16)
            nc.sync.dma_start(ident[:], ident_in[:])
            dstloc = cpool.tile([128, nch], fp32)
            nc.sync.dma_start(dstloc[:], dl_in[:])

            # deg^{-1/2} from int32 counts:  1 / sqrt(cnt + 1)
            cnts = cpool.tile([128, BLK], i32)
            nc.sync.dma_start(cnts[:], cnt_in[:])
            degf = cpool.tile([128, BLK], fp32)
            nc.vector.tensor_copy(degf[:], cnts[:])
            sq = cpool.tile([128, BLK], fp32)
            nc.scalar.activation(sq[:], degf[:], mybir.ActivationFunctionType.Sqrt,
                                 bias=1.0)
            dq = cpool.tile([128, BLK], fp32)
            nc.vector.reciprocal(dq[:], sq[:])

            # flat sqrt(deg) [1, SH] for the rank-1 bias fold (slice at
            # partition 0), plus b as [1,128] bf16 rows
            sqT = cpool.tile([1, SH], bf16)
            with tc.tile_pool(name="tmpf", bufs=1) as tpool:
                fw = SH // 8
                for j in range(8):
                    cf = tpool.tile([1, fw], i32, tag="cf")
                    nc.sync.dma_start(cf[:], cntf_in[0:1, j * fw:(j + 1) * fw])
                    dg = tpool.tile([1, fw], fp32, tag="dg")
                    nc.vector.tensor_copy(dg[:], cf[:])
                    nc.scalar.activation(sqT[0:1, j * fw:(j + 1) * fw], dg[:],
                                         mybir.ActivationFunctionType.Sqrt,
                                         bias=1.0)
            b1r = cpool.tile([1, 128], bf16)
            nc.vector.tensor_copy(b1r[:], b1t[0:1, :])
            b2r = cpool.tile([1, 128], bf16)
            nc.vector.tensor_copy(b2r[:], b2t[0:1, :])

            # DRAM bounce + gathered tables
            g1_b = dram.tile([SH, D], bf16, name="g1_b")
            g1_full = dram.tile([NP_, D], bf16, name="g1_full", addr_space="Shared")
            g2_b = dram.tile([SH, D], bf16, name="g2_b")
            g2_full = dram.tile([NP_, D], bf16, name="g2_full", addr_space="Shared")

            def own_piece(g_b, p):
                t = fpool.tile([128, PIECE, 128], bf16, tag="gown")
                nc.sync.dma_start(
                    t[:], g_b[:].rearrange("(a p) d -> p a d", p=128)
                    [:, p * PIECE:(p + 1) * PIECE, :])
                return t

            # ---- phase A: g1 = dq * (x @ W1) on own shard ---------------
            with nc.named_scope("phaseA"):
                for pc in range(NPIECE):
                    c0 = pc * PIECE
                    xc = xpool.tile([128, PIECE * 128], fp32, tag="xc")
                    nc.sync.dma_start(
                        xc[:], xT_in[:, c0 * 128:(c0 + PIECE) * 128])
                    xcb = xpool.tile([128, PIECE * 128], bf16, tag="xcb")
                    nc.vector.tensor_copy(xcb[:], xc[:])
                    gt = xpool.tile([128, PIECE, 128], bf16, tag="gt")
                    for j in range(PIECE):
                        ph = ps_w.tile([128, 128], fp32, space="PSUM", tag="ph")
                        nc.tensor.matmul(
                            out=ph[:], lhsT=xcb[:, j * 128:(j + 1) * 128],
                            rhs=w1b[:], start=True, stop=True)
                        nc.scalar.activation(
                            gt[:, j, :], ph[:],
                            mybir.ActivationFunctionType.Copy,
                            scale=dq[:, c0 + j:c0 + j + 1])
                    nc.sync.dma_start(
                        g1_b[:].rearrange("(a p) d -> p a d", p=128)
                        [:, c0:c0 + PIECE, :], gt[:])

            with nc.named_scope("ag1"):
                nc.gpsimd.collective_compute(
                    "AllGather", mybir.AluOpType.bypass,
                    replica_groups=[list(range(NCORES))],
                    ins=[g1_b[:].opt()], outs=[g1_full[:].opt()])

            # ---- aggregation over edges (shared for both layers) --------
            def aggregate(g_full, g_b, b_row, finalize, stage_dt, flush):
                for p in range(NPIECE):
                    pw = NB * pidx // 16
                    ix = ixpool.tile([128, pw], i16, tag="ix")
                    nc.sync.dma_start(ix[:], idx_in[:, p * pw:(p + 1) * pw])
                    msgs = []
                    for r in range(NB):
                        msg = mpool.tile([128, pidx // 128, 128], bf16,
                                         tag=f"msg{r}")
                        nc.gpsimd.dma_gather(
                            msg[:], g_full[r * BKT:(r + 1) * BKT, :],
                            ix[:, r * pidx // 16:(r + 1) * pidx // 16],
                            pidx, pidx, 128, single_packet=False,
                            queue_num=r)
                        msgs.append(msg)
                    gown = own_piece(g_b, p)
                    stage = fpool.tile([128, PIECE, 128], stage_dt, tag="stage")
                    for s in range(PIECE):
                        c = p * PIECE + s
                        pseg = ps_seg.tile([128, 128], fp32, space="PSUM",
                                           tag="pseg")
                        for r in range(NB):
                            gc0 = ((p * NB + r) * PIECE + s) * ck
                            st = stpool.tile([128, cap], bf16, tag="st")
                            nc.vector.tensor_tensor(
                                st[:].rearrange("q (c j) -> q c j", c=ck),
                                iota[:].rearrange("q (c j) -> q c j", c=ck),
                                dstloc[:, gc0:gc0 + ck].to_broadcast(
                                    [128, ck, 128]),
                                mybir.AluOpType.is_equal)
                            for k in range(ck):
                                nc.tensor.matmul(
                                    out=pseg[:],
                                    lhsT=st[:, k * 128:(k + 1) * 128],
                                    rhs=msgs[r][:, s * ck + k, :],
                                    start=(r == 0 and k == 0),
                                    stop=False)
                        # self-loop: pseg += I.T @ g_own ;  bias (pre-scale):
                        # pseg += sqrt(deg) (x) b  so that (pseg)*dq = out+b
                        nc.tensor.matmul(out=pseg[:], lhsT=ident[:],
                                         rhs=gown[:, s, :],
                                         start=False, stop=False)
                        nc.tensor.matmul(out=pseg[:], lhsT=sqT[0:1, c * 128:(c + 1) * 128],
                                         rhs=b_row[:],
                                         start=False, stop=True)
                        finalize(c, pseg[:], stage[:, s, :])
                    flush(p, stage)

            # ---- layer-1 finalize: h2 path, produces g2 -----------------
            def fin1(c, pslice, sl):
                r1 = fpool.tile([128, 128], bf16, tag="r1")
                nc.scalar.activation(r1[:], pslice,
                                     mybir.ActivationFunctionType.Lrelu,
                                     scale=dq[:, c:c + 1], alpha=NEG)
                ptr = ps_t.tile([128, 128], bf16, space="PSUM", tag="ptr")
                nc.tensor.transpose(ptr[:], r1[:], ident[:])
                r1T = fpool.tile([128, 128], bf16, tag="r1T")
                nc.vector.tensor_copy(r1T[:], ptr[:])
                ph2 = ps_w.tile([128, 128], fp32, space="PSUM", tag="ph")
                nc.tensor.matmul(out=ph2[:], lhsT=r1T[:], rhs=w2b[:],
                                 start=True, stop=True)
                nc.scalar.activation(sl, ph2[:],
                                     mybir.ActivationFunctionType.Copy,
                                     scale=dq[:, c:c + 1])

            def flush1(p, stage):
                nc.sync.dma_start(
                    g2_b[:].rearrange("(a p) d -> p a d", p=128)
                    [:, p * PIECE:(p + 1) * PIECE, :], stage[:])

            def fin2(c, pslice, sl):
                nc.scalar.activation(sl, pslice,
                                     mybir.ActivationFunctionType.Lrelu,
                                     scale=dq[:, c:c + 1], alpha=NEG)

            def flush2(p, stage):
                nc.sync.dma_start(
                    out_t[:].rearrange("(a p) d -> p a d", p=128)
                    [:, p * PIECE:(p + 1) * PIECE, :], stage[:])

            with nc.named_scope("agg1"):
                aggregate(g1_full, g1_b, b1r, fin1, bf16, flush1)

            with nc.named_scope("ag2"):
                nc.gpsimd.collective_compute(
                    "AllGather", mybir.AluOpType.bypass,
                    replica_groups=[list(range(NCORES))],
                    ins=[g2_b[:].opt()], outs=[g2_full[:].opt()])

            with nc.named_scope("agg2"):
                aggregate(g2_full, g2_b, b2r, fin2, fp32, flush2)

    nc.compile()
    return nc


def _preprocess(x, edge_index):
    src = np.asarray(edge_index[0], dtype=np.int64)
    dst = np.asarray(edge_index[1], dtype=np.int64)
    core = dst // SH
    block = (dst % SH) // 128
    dstloc = (dst % 128).astype(np.float32)
    bucket = src // BKT
    srcloc = (src % BKT).astype(np.int16)

    pc = block // PIECE
    sp = block % PIECE
    seg = ((core * NPIECE + pc) * NB + bucket) * PIECE + sp
    counts = np.bincount(seg, minlength=NCORES * NB * BLK)
    cap = max(640, int(-(-counts.max() // 128) * 128))

    order = np.argsort(seg, kind="stable")
    seg_s = seg[order]
    starts = np.zeros(NCORES * NB * BLK + 1, np.int64)
    np.cumsum(counts, out=starts[1:])
    pos = np.arange(E, dtype=np.int64) - starts[seg_s]
    slot = seg_s * cap + pos

    total = NCORES * NB * BLK * cap
    idx_arr = np.zeros(total, np.int16)
    idx_arr[slot] = srcloc[order]
    dl_arr = np.full(total, 255.0, np.float32)
    dl_arr[slot] = dstloc[order]
    ts = NB * BLK * cap
    idx_arr = idx_arr.reshape(NCORES, ts)
    dl_arr = dl_arr.reshape(NCORES, ts)

    cnt = np.bincount(dst, minlength=NP_).astype(np.int32)

    xpad = np.zeros((NP_, D), np.float32)
    xpad[:N] = x

    iota = np.tile(np.arange(128, dtype=np.float32), (128, cap // 128)) \
        .astype(ml_dtypes.bfloat16)
    ident = np.eye(128, dtype=ml_dtypes.bfloat16)

    return cap, idx_arr, dl_arr, cnt, xpad, iota, ident


def kernel(x, W1, b1, W2, b2, edge_index, batch):
    x = np.asarray(x, np.float32)
    W1 = np.asarray(W1, np.float32)
    W2 = np.asarray(W2, np.float32)
    b1 = np.asarray(b1, np.float32)
    b2 = np.asarray(b2, np.float32)

    cap, idx_arr, dl_arr, cnt, xpad, iota, ident = _preprocess(x, edge_index)

    if cap not in _CACHE:
        _CACHE[cap] = _build(cap)
    nc = _CACHE[cap]

    b1t = np.tile(b1, (128, 1))
    b2t = np.tile(b2, (128, 1))
    in_maps = []
    for c in range(NCORES):
        sl = slice(c * SH, (c + 1) * SH)
        wrapped = np.tile(idx_arr[c].reshape(-1, 16).T, (8, 1))
        in_maps.append({
            "xT": np.ascontiguousarray(xpad[sl].T),
            "w1": W1, "w2": W2, "b1t": b1t, "b2t": b2t,
            "cnt": np.ascontiguousarray(cnt[sl].reshape(BLK, 128).T),
            "cntf": np.ascontiguousarray(cnt[sl][None, :]),
            "idx": np.ascontiguousarray(wrapped),
            "dstloc": np.ascontiguousarray(dl_arr[c].reshape(-1, 128).T),
            "iota": iota, "ident": ident,
        })

    import os
    trace = bool(os.environ.get("KERNEL_TRACE"))
    rr = run_bass_kernel_spmd(nc, in_maps, list(range(NCORES)), trace=trace)
    if trace:
        kernel.last_results = rr
    out = np.concatenate([rr.results[c]["out"] for c in range(NCORES)], axis=0)
    return np.ascontiguousarray(out[:N])
